# revision 1
# baseline (speedup 1.0000x reference)
"""Trainium2 Bass kernel for nn_Bengio03biLm (bidirectional windowed LM).

Strategy: data-parallel over batch (16 seqs -> 2 per NeuronCore x 8 cores),
weights replicated. Per core, everything is computed on-chip:

  - The windowed "left/right context" projections are computed as 4 shifted
    [128,128]x[128,512] matmuls against a feature-major (transposed, padded)
    copy of the input prepared on host, so no [B,S,W*H] gather is ever
    materialized.
  - Activations flow token-major ([tokens, H] tiles); LayerNorm stats use
    bn_stats/bn_aggr; the LN gain/beta and biases are folded on host
    (W1' = g*W1, b1' = beta@W1 + b1).
  - The matmul inputs are re-transposed on the TensorEngine (128x128
    transposes via an identity operand).
  - The residual add and (when nonzero) biases are folded into the FF2
    PSUM accumulation via identity / ones-row matmuls.
  - All matmuls run in float32r (TF32-like, ~1e-4 rel err, ~2x bf16 rate).

Outputs: all_layers [L,S,B,2H] assembled on host from per-core token-major
tiles; last_layers is a host transpose of all_layers[-1].
"""
import numpy as np
import concourse.bacc as bacc
import concourse.tile as tile
import concourse.mybir as mybir
from concourse.bass_utils import run_bass_kernel_spmd

F32R = mybir.dt.float32r
F32 = mybir.dt.float32
AF = mybir.ActivationFunctionType
OP = mybir.AluOpType

B, S, H, W, L = 16, 2048, 512, 4, 2
KC = H // 128
N_CORES = 8
EPS = 1e-5


def _build_kernel(n_seq=2, n_chunks=8, T=256, skip_bias=False, uniform_b1=False):
    NT = T // 128
    SP = n_chunks * T
    nc = bacc.Bacc("TRN2", target_bir_lowering=False, debug=False)

    X = nc.dram_tensor("x", [n_seq, 128, KC, SP + 2 * W], F32R, kind="ExternalInput").ap()
    WP = nc.dram_tensor("wp", [128, 2, W, KC, H], F32R, kind="ExternalInput").ap()
    W1 = nc.dram_tensor("w1", [128, 2, L, KC, KC, 128], F32R, kind="ExternalInput").ap()
    W2 = nc.dram_tensor("w2", [128, 2, L, KC, H], F32R, kind="ExternalInput").ap()
    B1 = nc.dram_tensor("b1", [128, 2, L, KC], F32, kind="ExternalInput").ap()
    B2 = nc.dram_tensor("b2", [1, 2, L, H], F32R, kind="ExternalInput").ap()
    BP = nc.dram_tensor("bp", [1, 2, H], F32R, kind="ExternalInput").ap()
    IDN = nc.dram_tensor("ident", [128, 128], F32R, kind="ExternalInput").ap()
    ONES = nc.dram_tensor("ones", [1, 128], F32R, kind="ExternalInput").ap()
    OUT = nc.dram_tensor("out", [L, n_seq, SP, 2 * H], F32R, kind="ExternalOutput").ap()

    with tile.TileContext(nc) as tc:
        with (
            tc.tile_pool(name="singles", bufs=1) as singles,
            tc.tile_pool(name="xp", bufs=3) as xp,
            tc.tile_pool(name="state", bufs=3) as state_pool,
            tc.tile_pool(name="zp", bufs=2) as zp,
            tc.tile_pool(name="small", bufs=6) as small,
            tc.tile_pool(name="ps", bufs=2, space="PSUM") as ps,
        ):
            wp_sb = singles.tile([128, 2, W, KC, H], F32R)
            w1_sb = singles.tile([128, 2, L, KC, KC, 128], F32R)
            w2_sb = singles.tile([128, 2, L, KC, H], F32R)
            b1_sb = singles.tile([128, 2, L, KC], F32)
            b2_sb = singles.tile([1, 2, L, H], F32R)
            bp_sb = singles.tile([1, 2, H], F32R)
            id_sb = singles.tile([128, 128], F32R)
            ones_sb = singles.tile([1, 128], F32R)
            zero_sb = singles.tile([128, 1], F32)
            eps_sb = singles.tile([128, 1], F32)
            nc.sync.dma_start(wp_sb[:], WP[:])
            nc.sync.dma_start(w1_sb[:], W1[:])
            nc.sync.dma_start(w2_sb[:], W2[:])
            nc.sync.dma_start(b1_sb[:], B1[:])
            nc.sync.dma_start(b2_sb[:], B2[:])
            nc.sync.dma_start(bp_sb[:], BP[:])
            nc.sync.dma_start(id_sb[:], IDN[:])
            nc.sync.dma_start(ones_sb[:], ONES[:])
            nc.vector.memset(zero_sb[:], 0.0)
            nc.vector.memset(eps_sb[:], EPS)

            def ln_finish(mv, name):
                rstd = small.tile([128, NT], F32, name=f"rstd_{name}", tag="rstd")
                mrs = small.tile([128, NT], F32, name=f"mrs_{name}", tag="mrs")
                nc.scalar.activation(rstd[:], mv[:, :, 1], AF.Sqrt, bias=eps_sb[:], scale=1.0)
                nc.vector.reciprocal(rstd[:], rstd[:])
                nc.vector.tensor_tensor(mrs[:], mv[:, :, 0], rstd[:], OP.mult)
                return rstd, mrs

            for s in range(n_seq):
                for ch in range(n_chunks):
                    t0 = ch * T
                    xsb = xp.tile([128, KC, T + 2 * W], F32R, name="xsb")
                    nc.sync.dma_start(xsb[:], X[s, :, :, t0:t0 + T + 2 * W])

                    state = {}
                    for side in range(2):
                        lo = state_pool.tile([128, NT, H], F32R, name=f"lo{side}", tag=f"lo{side}")
                        st6 = small.tile([128, NT, 6], F32, name=f"st6p{side}", tag="st6")
                        mv = small.tile([128, NT, 2], F32, name=f"mvp{side}", tag="mv")
                        for tt in range(NT):
                            pp = ps.tile([128, H], F32, name="pp", tag="pp")
                            first = True
                            for j in range(W):
                                off = tt * 128 + j + (0 if side == 0 else W + 1)
                                for ki in range(KC):
                                    nc.tensor.matmul(
                                        pp[:], xsb[:, ki, off:off + 128],
                                        wp_sb[:, side, j, ki, :],
                                        start=first, stop=(skip_bias and j == W - 1 and ki == KC - 1))
                                    first = False
                            if not skip_bias:
                                nc.tensor.matmul(pp[:], ones_sb[:], bp_sb[:, side, :],
                                                 start=False, stop=True)
                            nc.vector.tensor_scalar(
                                lo[:, tt, :], pp[:], 0.0, None, OP.max)
                            nc.vector.bn_stats(st6[:, tt, :], lo[:, tt, :].bitcast(F32))
                            nc.vector.bn_aggr(mv[:, tt, :], st6[:, tt, :])
                        rstd, mrs = ln_finish(mv, f"p{side}")
                        state[side] = (lo, rstd, mrs)

                    for layer in range(L):
                        for side in range(2):
                            lo, rstd, mrs = state[side]
                            z = zp.tile([128, NT, H], F32R, name="z", tag="z")
                            for tt in range(NT):
                                nc.vector.tensor_scalar(
                                    z[:, tt, :], lo[:, tt, :].bitcast(F32),
                                    rstd[:, tt:tt + 1], mrs[:, tt:tt + 1],
                                    OP.mult, OP.subtract)
                            zft = zp.tile([128, KC, T], F32R, name="zft", tag="zft")
                            for kp in range(2):
                                ptr = ps.tile([128, 2, T], F32R, name="ptr", tag="ptr")
                                for k2 in range(2):
                                    ki = kp * 2 + k2
                                    for tt in range(NT):
                                        nc.tensor.transpose(
                                            ptr[:, k2, tt * 128:(tt + 1) * 128],
                                            z[:, tt, ki * 128:(ki + 1) * 128], id_sb[:])
                                if kp == 0:
                                    nc.vector.tensor_copy(zft[:, 0:2, :], ptr[:])
                                else:
                                    nc.scalar.copy(zft[:, 2:4, :], ptr[:])
                            h1 = zp.tile([128, KC, T], F32R, name="h1", tag="h1")
                            for mp in range(2):
                                pf = ps.tile([128, 2, T], F32, name="pf", tag="pf")
                                for m2 in range(2):
                                    mo = mp * 2 + m2
                                    for ki in range(KC):
                                        nc.tensor.matmul(
                                            pf[:, m2, :], w1_sb[:, side, layer, ki, mo, :],
                                            zft[:, ki, :],
                                            start=(ki == 0), stop=(ki == KC - 1))
                                if uniform_b1:
                                    nc.scalar.activation(
                                        h1[:, mp * 2:(mp + 1) * 2, :], pf[:], AF.Relu,
                                        bias=zero_sb[:], scale=1.0)
                                else:
                                    for m2 in range(2):
                                        mo = mp * 2 + m2
                                        nc.scalar.activation(
                                            h1[:, mo, :], pf[:, m2, :], AF.Relu,
                                            bias=b1_sb[:, side, layer, mo:mo + 1], scale=1.0)
                            lo_new = state_pool.tile([128, NT, H], F32R,
                                                     name=f"lon{side}", tag=f"lo{side}")
                            st6n = small.tile([128, NT, 6], F32, name="st6n", tag="st6")
                            mvn = small.tile([128, NT, 2], F32, name="mvn", tag="mv")
                            for tt in range(NT):
                                pf2 = ps.tile([128, H], F32, name="pf2", tag="pf2")
                                for ki in range(KC):
                                    nc.tensor.matmul(
                                        pf2[:], h1[:, ki, tt * 128:(tt + 1) * 128],
                                        w2_sb[:, side, layer, ki, :],
                                        start=(ki == 0), stop=False)
                                if not skip_bias:
                                    nc.tensor.matmul(pf2[:], ones_sb[:],
                                                     b2_sb[:, side, layer, :],
                                                     start=False, stop=False)
                                nc.tensor.matmul(pf2[:], id_sb[:], lo[:, tt, :],
                                                 start=False, stop=True)
                                if layer < L - 1:
                                    nc.vector.bn_stats(st6n[:, tt, :], pf2[:])
                                    nc.vector.bn_aggr(mvn[:, tt, :], st6n[:, tt, :])
                                nc.scalar.activation(lo_new[:, tt, :], pf2[:], AF.Copy)
                                nc.sync.dma_start(
                                    OUT[layer, s, t0 + tt * 128:t0 + (tt + 1) * 128,
                                        side * H:(side + 1) * H],
                                    lo_new[:, tt, :])
                            if layer < L - 1:
                                rstd_n, mrs_n = ln_finish(mvn, f"n{side}")
                                state[side] = (lo_new, rstd_n, mrs_n)
    nc.compile()
    return nc


def _prep_weights(Wl, bl, Wr, br, lw1, lb1, lw2, lb2, lg, lbeta,
                  rw1, rb1, rw2, rb2, rg, rbeta):
    f32 = np.float32
    wp = np.stack([Wl, Wr], axis=0).reshape(2, W, KC, 128, H)
    wp = np.ascontiguousarray(wp.transpose(3, 0, 1, 2, 4)).astype(f32)

    w1p = np.stack([lg[:, :, None] * lw1, rg[:, :, None] * rw1], axis=0)
    w1p = w1p.reshape(2, L, KC, 128, KC, 128)
    w1p = np.ascontiguousarray(w1p.transpose(3, 0, 1, 2, 4, 5)).astype(f32)

    w2p = np.stack([lw2, rw2], axis=0).reshape(2, L, KC, 128, H)
    w2p = np.ascontiguousarray(w2p.transpose(3, 0, 1, 2, 4)).astype(f32)

    b1p = np.stack([
        np.einsum('lh,lhk->lk', lbeta, lw1) + lb1,
        np.einsum('lh,lhk->lk', rbeta, rw1) + rb1], axis=0)
    b1p = np.ascontiguousarray(b1p.reshape(2, L, KC, 128).transpose(3, 0, 1, 2)).astype(f32)

    b2p = np.stack([lb2, rb2], axis=0)[None].astype(f32)
    bpp = np.stack([bl, br], axis=0)[None].astype(f32)
    return wp, w1p, w2p, b1p, b2p, bpp


def _prep_x(x_seqs, left_padding, right_padding):
    """x_seqs [n_seq, S, H] -> feature-major padded [n_seq, 128, KC, S+2W]."""
    n_seq = x_seqs.shape[0]
    xpad = np.concatenate([
        np.broadcast_to(left_padding, (n_seq, W, H)), x_seqs,
        np.broadcast_to(right_padding, (n_seq, W, H))], axis=1)
    return np.ascontiguousarray(
        xpad.transpose(0, 2, 1).reshape(n_seq, KC, 128, S + 2 * W)
        .transpose(0, 2, 1, 3)).astype(np.float32)


def kernel(inputs, left_padding, right_padding, Wl, bl, Wr, br,
           lw1, lb1, lw2, lb2, lg, lbeta,
           rw1, rb1, rw2, rb2, rg, rbeta):
    inputs = np.asarray(inputs, np.float32)
    args = [np.asarray(a, np.float32) for a in
            (left_padding, right_padding, Wl, bl, Wr, br,
             lw1, lb1, lw2, lb2, lg, lbeta, rw1, rb1, rw2, rb2, rg, rbeta)]
    (left_padding, right_padding, Wl, bl, Wr, br,
     lw1, lb1, lw2, lb2, lg, lbeta, rw1, rb1, rw2, rb2, rg, rbeta) = args

    wp, w1p, w2p, b1p, b2p, bpp = _prep_weights(
        Wl, bl, Wr, br, lw1, lb1, lw2, lb2, lg, lbeta,
        rw1, rb1, rw2, rb2, rg, rbeta)
    skip_bias = (not bpp.any()) and (not b2p.any())
    uniform_b1 = not b1p.any()

    n_seq = B // N_CORES
    nc = _build_kernel(n_seq=n_seq, n_chunks=S // 256, T=256,
                       skip_bias=skip_bias, uniform_b1=uniform_b1)

    common = {
        "wp": wp, "w1": w1p, "w2": w2p, "b1": b1p, "b2": b2p, "bp": bpp,
        "ident": np.eye(128, dtype=np.float32),
        "ones": np.ones((1, 128), dtype=np.float32),
    }
    in_maps = []
    for c in range(N_CORES):
        xs = inputs[c * n_seq:(c + 1) * n_seq]
        in_maps.append({"x": _prep_x(xs, left_padding, right_padding), **common})

    res = run_bass_kernel_spmd(nc, in_maps, core_ids=list(range(N_CORES)))

    all_layers = np.empty((L, S, B, 2 * H), np.float32)
    for c in range(N_CORES):
        out_c = res.results[c]["out"]  # [L, n_seq, S, 2H]
        all_layers[:, :, c * n_seq:(c + 1) * n_seq, :] = out_c.transpose(0, 2, 1, 3)
    last_layers = np.ascontiguousarray(all_layers[L - 1].transpose(1, 0, 2))
    return all_layers, last_layers
